# revision 16
# baseline (speedup 1.0000x reference)
"""FAVOR+ causal linear attention (relu-kernel Performer) on 8 TRN2 NeuronCores.

Problem: B=2, L=4096, H=8, D=64, M=128, fp32. 16 (b,h) pairs -> 2 per core.

Software-pipelined emission, 4 stages deep; per iteration `it`:
  DCOPY(it-2) delta copies (ACT) + kv tree (Pool)
  FEAT(it)    feature matmuls (PE) + relu (ACT)
  MID(it-1)   kp transposes + 2x bf16 copy, st matmuls + mask (PE/DVE)
  NUM(it-2)   carry/delta/st matmuls into num+den PSUM (PE)
  DIV(it-2)   fused reciprocal + broadcast multiply + store (DVE/DMA)
Each engine's per-iteration work depends only on >=1-iteration-old tiles, so
the ~6us cross-engine chain of one SC does not set the cadence; the busiest
engine does (ACT, ~2.9us/SC, saturated in steady state).

Key structures vs the 37202ns predecessor:
  - kv prefix state in SBUF bf16, advanced once per SC by the Pool engine
    (pairwise tree over the four sub-chunk deltas) - no serial prefix chain.
  - cross-sub prefix terms are extra N=65 matmuls against the deltas.
  - den is computed by separate N=1 matmul groups into a tiny PSUM tile
    (nearly free on PE), which makes the num tile exactly one bank for BOTH
    pairs -> ONE fused reciprocal + ONE fused output multiply per SC on DVE
    (DVE 3016 -> 2765 ns/SC, dropping it below the ACT pacer).
  - single fused store for the last SC; its unused kv tree is skipped.
PSUM = 8 banks exactly: features 2x2-bank slots + a 4x1-bank rotation
shared by kp/st/d/num/den tiles. Further: fused dsb SBUF tile feeds a
4-op Pool tree; pair-0 deltas are emitted before num so ACT's d copies
never wait at iteration start; the first qk loads are split per-SC so
the opening feature matmuls wait on half-size transfers (first two
qk halves + all v loads split per-SC).
34803 ns TimelineSim (was 37202).
"""

import math

import numpy as np
import ml_dtypes

import concourse.bass as bass
import concourse.bacc as bacc
import concourse.mybir as mybir
import concourse.tile as tile
from concourse.bass_utils import run_bass_kernel_spmd
from concourse.masks import make_identity

F32 = mybir.dt.float32
BF16 = mybir.dt.bfloat16

B, L, H, D, M = 2, 4096, 8, 64, 128
NCORES = 8
NPAIR = (B * H) // NCORES
SC = 512
NSUB = SC // 128
NSC = L // SC
DV = D + 1
RATIO = 1.0 / math.sqrt(M)

_NC_CACHE = {}


def build_nc():
    nc = bacc.Bacc("TRN2", target_bir_lowering=False, debug=False)
    qk2 = nc.dram_tensor("qk2", [NPAIR, D, 2, L], BF16, kind="ExternalInput").ap()
    v = nc.dram_tensor("v", [NPAIR, 128, NSC, NSUB, DV], BF16, kind="ExternalInput").ap()
    constsd = nc.dram_tensor("consts", [128, 256], BF16, kind="ExternalInput").ap()
    out = nc.dram_tensor("out", [NSC, 128, NPAIR, NSUB, D], BF16, kind="ExternalOutput").ap()

    with tile.TileContext(nc) as tc:
        with (
            tc.tile_pool(name="const", bufs=1) as cpool,
            tc.tile_pool(name="io", bufs=3) as iopool,
            tc.tile_pool(name="feat", bufs=2) as fpool,
            tc.tile_pool(name="ps_qk", bufs=2, space="PSUM") as ps_qk,
            tc.tile_pool(name="ps_m", bufs=4, space="PSUM") as ps_m,
        ):
            consts = cpool.tile([128, 256], BF16)
            mask = consts[:, 0:128]
            ptile = consts[0:D, 128:256]
            ident = cpool.tile([128, 128], BF16)
            mask_b = bass.AP(
                tensor=mask.tensor,
                offset=mask.offset,
                ap=[mask.ap[0], [0, NSUB], mask.ap[1]],
            )
            nc.sync.dma_start(out=consts, in_=constsd)
            make_identity(nc, ident)

            # cross-iteration state (python lists indexed by sc)
            qk2_sb = [None] * (NSC // 2)
            v_sb = [None] * (NSC // 2)
            out2_sb = [None] * (NSC // 2)
            fT = [[None] * NPAIR for _ in range(NSC)]
            kp_sb = [None] * NSC
            st_sb = [None] * NSC
            d_sb = [[None] * NPAIR for _ in range(NSC)]
            kv_sb = [None] * NSC  # kv state AFTER sc
            num_ps = [[None] * NPAIR for _ in range(NSC)]

            def emit_feat(n):
                half = n // 2
                off = (n % 2) * SC
                if n % 2 == 0:
                    t0 = n * SC
                    qkt2 = iopool.tile(
                        [D, NPAIR, 2, 2 * SC], BF16, tag="qkt", name=f"qkt_{half}"
                    )
                    if n <= 2:
                        # split loads so feat matmuls wait on half-size
                        # transfers
                        for pair in range(NPAIR):
                            nc.sync.dma_start(
                                out=qkt2[:, pair, :, 0:SC],
                                in_=qk2[pair, :, :, t0 : t0 + SC],
                            )
                        for pair in range(NPAIR):
                            nc.sync.dma_start(
                                out=qkt2[:, pair, :, SC : 2 * SC],
                                in_=qk2[pair, :, :, t0 + SC : t0 + 2 * SC],
                            )
                    else:
                        for pair in range(NPAIR):
                            nc.sync.dma_start(
                                out=qkt2[:, pair], in_=qk2[pair, :, :, t0 : t0 + 2 * SC]
                            )
                    qk2_sb[half] = qkt2
                    vt2 = iopool.tile(
                        [128, NPAIR, 2, NSUB, DV], BF16, tag="vt", name=f"vt_{half}"
                    )
                    if n == 0:
                        for scp in range(2):
                            nc.sync.dma_start(
                                out=vt2[:, :, scp],
                                in_=v[:, :, scp].rearrange("p r u d -> r p u d"),
                            )
                    else:
                        nc.sync.dma_start(
                            out=vt2,
                            in_=v[:, :, n : n + 2].rearrange("p r s u d -> r p s u d"),
                        )
                    v_sb[half] = vt2
                qkt = qk2_sb[half]

                def feat_pair(pair):
                    qk_ps = ps_qk.tile(
                        [M, 2, SC], F32, tag="qk", name=f"qkp_{pair}_{n}"
                    )
                    nc.tensor.matmul(
                        qk_ps[:, 0], ptile, qkt[:, pair, 0, off : off + SC],
                        start=True, stop=True,
                    )
                    nc.tensor.matmul(
                        qk_ps[:, 1], ptile, qkt[:, pair, 1, off : off + SC],
                        start=True, stop=True,
                    )
                    f = fpool.tile(
                        [M, 2, SC], BF16, tag="fT", bufs=8, name=f"fT_{pair}_{n}"
                    )
                    nc.scalar.activation(f, qk_ps, mybir.ActivationFunctionType.Relu)
                    fT[n][pair] = f

                return feat_pair

            def emit_transp(n):
                # kp transposes (PE) + 2x DVE copy
                kp_ps = ps_m.tile(
                    [128, NPAIR, NSUB, 128], BF16, tag="m", name=f"kpp_{n}"
                )
                for pair in range(NPAIR):
                    for s in range(NSUB):
                        sl = slice(s * 128, (s + 1) * 128)
                        nc.tensor.transpose(
                            kp_ps[:, pair, s], fT[n][pair][:, 1, sl], ident
                        )
                kp = fpool.tile(
                    [128, NPAIR, NSUB, 128], BF16, tag="kp", bufs=3, name=f"kp_{n}"
                )
                nc.vector.tensor_copy(kp, kp_ps)
                kp_sb[n] = kp

            def emit_mid_pe1(n):
                # st matmuls (PE) + mask (DVE)
                st = fpool.tile(
                    [128, NPAIR, NSUB, 128], BF16, tag="st", bufs=4, name=f"st_{n}"
                )
                for pair in range(NPAIR):
                    st_ps = ps_m.tile(
                        [128, NSUB, 128], F32, tag="m", name=f"stp_{pair}_{n}"
                    )
                    for s in range(NSUB):
                        sl = slice(s * 128, (s + 1) * 128)
                        nc.tensor.matmul(
                            st_ps[:, s], fT[n][pair][:, 1, sl], fT[n][pair][:, 0, sl],
                            start=True, stop=True,
                        )
                    nc.vector.tensor_tensor(
                        st[:, pair], st_ps, mask_b, mybir.AluOpType.mult
                    )
                st_sb[n] = st

            d_ps_l = [None] * NSC

            def emit_dmm(n, pair):
                # deltas (PE), per pair so pair 0 can precede num emission
                vt = v_sb[n // 2][:, :, n % 2]
                if pair == 0:
                    d_ps_l[n] = []
                d_ps = ps_m.tile(
                    [128, NSUB, DV], F32, tag="m", name=f"dp_{pair}_{n}"
                )
                for s in range(NSUB):
                    nc.tensor.matmul(
                        d_ps[:, s], kp_sb[n][:, pair, s], vt[:, pair, s],
                        start=True, stop=True,
                    )
                d_ps_l[n].append(d_ps)

            def emit_dcopy(n, on_dve=False):
                # d copies (ACT; DVE in the tail), kv tree (Pool)
                dts = d_ps_l[n]
                last = n == NSC - 1
                dsbf = fpool.tile(
                    [128, NPAIR, NSUB, DV], BF16, tag="dsb", bufs=4,
                    name=f"dsb_{n}",
                )
                for pair in range(NPAIR):
                    dsb = dsbf[:, pair]
                    if on_dve:
                        nc.vector.tensor_copy(dsb, dts[pair])
                    else:
                        nc.scalar.copy(out=dsb, in_=dts[pair])
                    d_sb[n][pair] = dsb
                if last:
                    return
                # Pool: kv(n) = kv(n-1) + ((d0+d1)+(d2+d3)) per pair
                t01 = fpool.tile([128, NPAIR, DV], BF16, tag="t01", bufs=2, name=f"t01_{n}")
                t23 = fpool.tile([128, NPAIR, DV], BF16, tag="t23", bufs=2, name=f"t23_{n}")
                ssum = fpool.tile([128, NPAIR, DV], BF16, tag="ss", bufs=2, name=f"ss_{n}")
                kv = fpool.tile([128, NPAIR, DV], BF16, tag="kv", bufs=4, name=f"kv_{n}")
                nc.gpsimd.tensor_tensor(
                    t01, dsbf[:, :, 0], dsbf[:, :, 1], mybir.AluOpType.add
                )
                nc.gpsimd.tensor_tensor(
                    t23, dsbf[:, :, 2], dsbf[:, :, 3], mybir.AluOpType.add
                )
                nc.gpsimd.tensor_tensor(ssum, t01, t23, mybir.AluOpType.add)
                if n == 0:
                    nc.gpsimd.tensor_copy(kv, ssum)
                else:
                    nc.gpsimd.tensor_tensor(
                        kv, kv_sb[n - 1], ssum, mybir.AluOpType.add
                    )
                kv_sb[n] = kv

            def emit_num(n):
                # num (cols 0:64) fused both pairs in ONE 1-bank tile; den via
                # separate N=1 matmul groups into a tiny den tile -> fused
                # reciprocal + fused out multiply on DVE.
                vt = v_sb[n // 2][:, :, n % 2]
                nps = ps_m.tile(
                    [128, NPAIR, NSUB, D], F32, tag="m", name=f"nump_{n}"
                )
                dps = ps_m.tile(
                    [128, NPAIR, NSUB, 1], F32, tag="m", name=f"denp_{n}"
                )
                num_ps[n][0] = nps
                num_ps[n][1] = dps
                for pair in range(NPAIR):
                    for s in range(NSUB):
                        sl = slice(s * 128, (s + 1) * 128)
                        qp_s = fT[n][pair][:, 0, sl]
                        ops = []
                        if n > 0:
                            ops.append(kv_sb[n - 1][:, pair])
                        for j in range(s):
                            ops.append(d_sb[n][pair][:, j])
                        for cols, ob in (
                            (slice(0, D), nps[:, pair, s]),
                            (slice(D, DV), dps[:, pair, s]),
                        ):
                            first = True
                            for rhs in ops:
                                nc.tensor.matmul(
                                    ob, qp_s, rhs[:, cols],
                                    start=first, stop=False,
                                )
                                first = False
                            nc.tensor.matmul(
                                ob, st_sb[n][:, pair, s], vt[:, pair, s, cols],
                                start=first, stop=True,
                            )

            def emit_div(n):
                half = n // 2
                if n % 2 == 0:
                    out2_sb[half] = iopool.tile(
                        [128, 2, NPAIR, NSUB, D], BF16, tag="out_sb",
                        name=f"out_sb_{half}",
                    )
                out_sb = out2_sb[half][:, n % 2]
                nps, dps = num_ps[n]
                recip = fpool.tile(
                    [128, NPAIR, NSUB], F32, tag="recip", bufs=2, name=f"recip_{n}"
                )
                nc.vector.reciprocal(recip, dps[:, :, :, 0])
                rc_b = bass.AP(
                    tensor=recip.tensor,
                    offset=recip.offset,
                    ap=[recip.ap[0], recip.ap[1], recip.ap[2], [0, D]],
                )
                nc.vector.tensor_tensor(
                    out_sb, nps, rc_b, mybir.AluOpType.mult
                )
                if n == NSC - 2:
                    nc.sync.dma_start(out=out[n], in_=out2_sb[half][:, 0])
                elif n == NSC - 1:
                    nc.sync.dma_start(out=out[n], in_=out2_sb[half][:, 1])
                elif n % 2 == 1:
                    nc.sync.dma_start(
                        out=out[n - 1 : n + 1].rearrange("s p a b c -> p s a b c"),
                        in_=out2_sb[half],
                    )

            for it in range(NSC + 3):
                if 2 <= it <= NSC + 1:
                    emit_dcopy(it - 2)
                fp = emit_feat(it) if it < NSC else None
                if fp is not None:
                    fp(0)
                if 1 <= it <= NSC:
                    emit_transp(it - 1)
                if fp is not None:
                    fp(1)
                if 1 <= it <= NSC:
                    emit_mid_pe1(it - 1)
                if 1 <= it <= NSC:
                    emit_dmm(it - 1, 0)
                if 2 <= it <= NSC + 1:
                    emit_num(it - 2)
                if 1 <= it <= NSC:
                    emit_dmm(it - 1, 1)
                if 2 <= it <= NSC + 1:
                    emit_div(it - 2)
    nc.compile()
    return nc


def _get_nc():
    if "nc" not in _NC_CACHE:
        _NC_CACHE["nc"] = build_nc()
    return _NC_CACHE["nc"]


def shard_inputs(query, key, value, projection_matrix):
    bf = ml_dtypes.bfloat16
    q = np.transpose(query, (0, 2, 3, 1)).reshape(B * H, D, L)
    k = np.transpose(key, (0, 2, 3, 1)).reshape(B * H, D, L)
    qk = np.stack([q, k], axis=2).astype(bf)  # [BH, D, 2, L]
    vv = np.transpose(value, (0, 2, 1, 3)).reshape(B * H, NSC, NSUB, 128, D)
    vv = np.transpose(vv, (0, 3, 1, 2, 4))  # [BH, 128, NSC, NSUB, D]
    vvp = np.ones((B * H, 128, NSC, NSUB, DV), dtype=bf)
    vvp[..., 0:D] = vv.astype(bf)
    km = np.arange(128)
    consts = np.zeros((128, 256), dtype=bf)
    consts[:, 0:128] = (km[:, None] <= km[None, :]).astype(bf)
    consts[0:D, 128:256] = (projection_matrix.T * RATIO).astype(bf)
    in_maps = []
    for c in range(NCORES):
        sl = slice(c * NPAIR, (c + 1) * NPAIR)
        in_maps.append(
            {
                "qk2": np.ascontiguousarray(qk[sl]),
                "v": np.ascontiguousarray(vvp[sl]),
                "consts": consts,
            }
        )
    return in_maps


def unshard_output(results):
    o = np.stack([np.asarray(r["out"], dtype=np.float32) for r in results], axis=0)
    o = o.transpose(0, 3, 1, 4, 2, 5).reshape(B, H, L, D).transpose(0, 2, 1, 3)
    return np.ascontiguousarray(o)


def kernel(query, key, value, projection_matrix, _trace=False):
    nc = _get_nc()
    in_maps = shard_inputs(
        np.asarray(query, dtype=np.float32),
        np.asarray(key, dtype=np.float32),
        np.asarray(value, dtype=np.float32),
        np.asarray(projection_matrix, dtype=np.float32),
    )
    res = run_bass_kernel_spmd(nc, in_maps, core_ids=list(range(NCORES)), trace=_trace)
    out = unshard_output(res.results)
    if _trace:
        return out, res
    return out


# revision 18
# speedup vs baseline: 1.0014x; 1.0014x over previous
"""FAVOR+ causal linear attention (relu-kernel Performer) on 8 TRN2 NeuronCores.

Problem: B=2, L=4096, H=8, D=64, M=128, fp32. 16 (b,h) pairs -> 2 per core.

Software-pipelined emission, 4 stages deep; per iteration `it`:
  DCOPY(it-2) delta copies (ACT) + kv tree (Pool)
  FEAT(it)    feature matmuls (PE) + relu (ACT)
  MID(it-1)   kp transposes + 2x bf16 copy, st matmuls + mask (PE/DVE)
  NUM(it-2)   carry/delta/st matmuls into num+den PSUM (PE)
  DIV(it-2)   fused reciprocal + broadcast multiply + store (DVE/DMA)
Each engine's per-iteration work depends only on >=1-iteration-old tiles, so
the ~6us cross-engine chain of one SC does not set the cadence; the busiest
engine does (ACT, ~2.9us/SC, saturated in steady state).

Key structures vs the 37202ns predecessor:
  - kv prefix state in SBUF bf16, advanced once per SC by the Pool engine
    (pairwise tree over the four sub-chunk deltas) - no serial prefix chain.
  - cross-sub prefix terms are extra N=65 matmuls against the deltas.
  - den is computed by separate N=1 matmul groups into a tiny PSUM tile
    (nearly free on PE), which makes the num tile exactly one bank for BOTH
    pairs -> ONE fused reciprocal + ONE fused output multiply per SC on DVE
    (DVE 3016 -> 2765 ns/SC, dropping it below the ACT pacer).
  - single fused store for the last SC; its unused kv tree is skipped.
PSUM = 8 banks exactly: features 2x2-bank slots + a 4x1-bank rotation
shared by kp/st/d/num/den tiles. Further: fused dsb SBUF tile feeds a
4-op Pool tree; pair-0 deltas are emitted before num so ACT's d copies
never wait at iteration start; the first qk loads are split per-SC so
the opening feature matmuls wait on half-size transfers (first two
qk halves + all v loads split per-SC).
34803 ns TimelineSim (was 37202).
"""

import math

import numpy as np
import ml_dtypes

import concourse.bass as bass
import concourse.bacc as bacc
import concourse.mybir as mybir
import concourse.tile as tile
from concourse.bass_utils import run_bass_kernel_spmd
from concourse.masks import make_identity

F32 = mybir.dt.float32
BF16 = mybir.dt.bfloat16

B, L, H, D, M = 2, 4096, 8, 64, 128
NCORES = 8
NPAIR = (B * H) // NCORES
SC = 512
NSUB = SC // 128
NSC = L // SC
DV = D + 1
RATIO = 1.0 / math.sqrt(M)

_NC_CACHE = {}


def build_nc():
    nc = bacc.Bacc("TRN2", target_bir_lowering=False, debug=False)
    qk2 = nc.dram_tensor("qk2", [NPAIR, D, 2, L], BF16, kind="ExternalInput").ap()
    v = nc.dram_tensor("v", [NPAIR, 128, NSC, NSUB, DV], BF16, kind="ExternalInput").ap()
    constsd = nc.dram_tensor("consts", [128, 256], BF16, kind="ExternalInput").ap()
    out = nc.dram_tensor("out", [NSC, 128, NPAIR, NSUB, D], BF16, kind="ExternalOutput").ap()

    with tile.TileContext(nc) as tc:
        with (
            tc.tile_pool(name="const", bufs=1) as cpool,
            tc.tile_pool(name="io", bufs=3) as iopool,
            tc.tile_pool(name="feat", bufs=2) as fpool,
            tc.tile_pool(name="ps_qk", bufs=2, space="PSUM") as ps_qk,
            tc.tile_pool(name="ps_m", bufs=4, space="PSUM") as ps_m,
        ):
            consts = cpool.tile([128, 256], BF16)
            mask = consts[:, 0:128]
            ptile = consts[0:D, 128:256]
            ident = cpool.tile([128, 128], BF16)
            mask_b = bass.AP(
                tensor=mask.tensor,
                offset=mask.offset,
                ap=[mask.ap[0], [0, NSUB], mask.ap[1]],
            )
            nc.sync.dma_start(out=consts, in_=constsd)
            make_identity(nc, ident)

            # cross-iteration state (python lists indexed by sc)
            qk2_sb = [None] * (NSC // 2)
            v_sb = [None] * (NSC // 2)
            out2_sb = [None] * (NSC // 2)
            fT = [[None] * NPAIR for _ in range(NSC)]
            kp_sb = [None] * NSC
            st_sb = [None] * NSC
            d_sb = [[None] * NPAIR for _ in range(NSC)]
            kv_sb = [None] * NSC  # kv state AFTER sc
            num_ps = [[None] * NPAIR for _ in range(NSC)]

            def emit_feat(n):
                half = n // 2
                off = (n % 2) * SC
                if n % 2 == 0:
                    t0 = n * SC
                    qkt2 = iopool.tile(
                        [D, NPAIR, 2, 2 * SC], BF16, tag="qkt", name=f"qkt_{half}"
                    )
                    if n <= 2:
                        # split loads so feat matmuls wait on half-size
                        # transfers
                        for pair in range(NPAIR):
                            nc.sync.dma_start(
                                out=qkt2[:, pair, :, 0:SC],
                                in_=qk2[pair, :, :, t0 : t0 + SC],
                            )
                        for pair in range(NPAIR):
                            nc.sync.dma_start(
                                out=qkt2[:, pair, :, SC : 2 * SC],
                                in_=qk2[pair, :, :, t0 + SC : t0 + 2 * SC],
                            )
                    else:
                        for pair in range(NPAIR):
                            nc.sync.dma_start(
                                out=qkt2[:, pair], in_=qk2[pair, :, :, t0 : t0 + 2 * SC]
                            )
                    qk2_sb[half] = qkt2
                    vt2 = iopool.tile(
                        [128, NPAIR, 2, NSUB, DV], BF16, tag="vt", name=f"vt_{half}"
                    )
                    if n <= 2:
                        for scp in range(2):
                            nc.sync.dma_start(
                                out=vt2[:, :, scp],
                                in_=v[:, :, n + scp].rearrange("p r u d -> r p u d"),
                            )
                    else:
                        nc.sync.dma_start(
                            out=vt2,
                            in_=v[:, :, n : n + 2].rearrange("p r s u d -> r p s u d"),
                        )
                    v_sb[half] = vt2
                qkt = qk2_sb[half]

                def feat_pair(pair):
                    qk_ps = ps_qk.tile(
                        [M, 2, SC], F32, tag="qk", name=f"qkp_{pair}_{n}"
                    )
                    nc.tensor.matmul(
                        qk_ps[:, 0], ptile, qkt[:, pair, 0, off : off + SC],
                        start=True, stop=True,
                    )
                    nc.tensor.matmul(
                        qk_ps[:, 1], ptile, qkt[:, pair, 1, off : off + SC],
                        start=True, stop=True,
                    )
                    f = fpool.tile(
                        [M, 2, SC], BF16, tag="fT", bufs=8, name=f"fT_{pair}_{n}"
                    )
                    nc.scalar.activation(f, qk_ps, mybir.ActivationFunctionType.Relu)
                    fT[n][pair] = f

                return feat_pair

            def emit_transp(n):
                # kp transposes (PE) + 2x DVE copy
                kp_ps = ps_m.tile(
                    [128, NPAIR, NSUB, 128], BF16, tag="m", name=f"kpp_{n}"
                )
                for pair in range(NPAIR):
                    for s in range(NSUB):
                        sl = slice(s * 128, (s + 1) * 128)
                        nc.tensor.transpose(
                            kp_ps[:, pair, s], fT[n][pair][:, 1, sl], ident
                        )
                kp = fpool.tile(
                    [128, NPAIR, NSUB, 128], BF16, tag="kp", bufs=3, name=f"kp_{n}"
                )
                nc.vector.tensor_copy(kp, kp_ps)
                kp_sb[n] = kp

            def emit_mid_pe1(n):
                # st matmuls (PE) + mask (DVE)
                st = fpool.tile(
                    [128, NPAIR, NSUB, 128], BF16, tag="st", bufs=4, name=f"st_{n}"
                )
                for pair in range(NPAIR):
                    st_ps = ps_m.tile(
                        [128, NSUB, 128], F32, tag="m", name=f"stp_{pair}_{n}"
                    )
                    for s in range(NSUB):
                        sl = slice(s * 128, (s + 1) * 128)
                        nc.tensor.matmul(
                            st_ps[:, s], fT[n][pair][:, 1, sl], fT[n][pair][:, 0, sl],
                            start=True, stop=True,
                        )
                    nc.vector.tensor_tensor(
                        st[:, pair], st_ps, mask_b, mybir.AluOpType.mult
                    )
                st_sb[n] = st

            d_ps_l = [None] * NSC

            def emit_dmm(n, pair):
                # deltas (PE), per pair so pair 0 can precede num emission
                vt = v_sb[n // 2][:, :, n % 2]
                if pair == 0:
                    d_ps_l[n] = []
                d_ps = ps_m.tile(
                    [128, NSUB, DV], F32, tag="m", name=f"dp_{pair}_{n}"
                )
                for s in range(NSUB):
                    nc.tensor.matmul(
                        d_ps[:, s], kp_sb[n][:, pair, s], vt[:, pair, s],
                        start=True, stop=True,
                    )
                d_ps_l[n].append(d_ps)

            def emit_dcopy(n, on_dve=False):
                # d copies (ACT; DVE in the tail), kv tree (Pool)
                dts = d_ps_l[n]
                last = n == NSC - 1
                dsbf = fpool.tile(
                    [128, NPAIR, NSUB, DV], BF16, tag="dsb", bufs=4,
                    name=f"dsb_{n}",
                )
                for pair in range(NPAIR):
                    dsb = dsbf[:, pair]
                    if on_dve:
                        nc.vector.tensor_copy(dsb, dts[pair])
                    else:
                        nc.scalar.copy(out=dsb, in_=dts[pair])
                    d_sb[n][pair] = dsb
                if last:
                    return
                # Pool: kv(n) = kv(n-1) + ((d0+d1)+(d2+d3)) per pair
                t01 = fpool.tile([128, NPAIR, DV], BF16, tag="t01", bufs=2, name=f"t01_{n}")
                t23 = fpool.tile([128, NPAIR, DV], BF16, tag="t23", bufs=2, name=f"t23_{n}")
                ssum = fpool.tile([128, NPAIR, DV], BF16, tag="ss", bufs=2, name=f"ss_{n}")
                kv = fpool.tile([128, NPAIR, DV], BF16, tag="kv", bufs=4, name=f"kv_{n}")
                nc.gpsimd.tensor_tensor(
                    t01, dsbf[:, :, 0], dsbf[:, :, 1], mybir.AluOpType.add
                )
                nc.gpsimd.tensor_tensor(
                    t23, dsbf[:, :, 2], dsbf[:, :, 3], mybir.AluOpType.add
                )
                nc.gpsimd.tensor_tensor(ssum, t01, t23, mybir.AluOpType.add)
                if n == 0:
                    nc.gpsimd.tensor_copy(kv, ssum)
                else:
                    nc.gpsimd.tensor_tensor(
                        kv, kv_sb[n - 1], ssum, mybir.AluOpType.add
                    )
                kv_sb[n] = kv

            def emit_num(n):
                # num (cols 0:64) fused both pairs in ONE 1-bank tile; den via
                # separate N=1 matmul groups into a tiny den tile -> fused
                # reciprocal + fused out multiply on DVE.
                vt = v_sb[n // 2][:, :, n % 2]
                nps = ps_m.tile(
                    [128, NPAIR, NSUB, D], F32, tag="m", name=f"nump_{n}"
                )
                dps = ps_m.tile(
                    [128, NPAIR, NSUB, 1], F32, tag="m", name=f"denp_{n}"
                )
                num_ps[n][0] = nps
                num_ps[n][1] = dps
                for pair in range(NPAIR):
                    for s in range(NSUB):
                        sl = slice(s * 128, (s + 1) * 128)
                        qp_s = fT[n][pair][:, 0, sl]
                        ops = []
                        if n > 0:
                            ops.append(kv_sb[n - 1][:, pair])
                        for j in range(s):
                            ops.append(d_sb[n][pair][:, j])
                        for cols, ob in (
                            (slice(0, D), nps[:, pair, s]),
                            (slice(D, DV), dps[:, pair, s]),
                        ):
                            first = True
                            for rhs in ops:
                                nc.tensor.matmul(
                                    ob, qp_s, rhs[:, cols],
                                    start=first, stop=False,
                                )
                                first = False
                            nc.tensor.matmul(
                                ob, st_sb[n][:, pair, s], vt[:, pair, s, cols],
                                start=first, stop=True,
                            )

            def emit_div(n):
                half = n // 2
                if n % 2 == 0:
                    out2_sb[half] = iopool.tile(
                        [128, 2, NPAIR, NSUB, D], BF16, tag="out_sb",
                        name=f"out_sb_{half}",
                    )
                out_sb = out2_sb[half][:, n % 2]
                nps, dps = num_ps[n]
                recip = fpool.tile(
                    [128, NPAIR, NSUB], F32, tag="recip", bufs=2, name=f"recip_{n}"
                )
                nc.vector.reciprocal(recip, dps[:, :, :, 0])
                rc_b = bass.AP(
                    tensor=recip.tensor,
                    offset=recip.offset,
                    ap=[recip.ap[0], recip.ap[1], recip.ap[2], [0, D]],
                )
                nc.vector.tensor_tensor(
                    out_sb, nps, rc_b, mybir.AluOpType.mult
                )
                if n == NSC - 2:
                    nc.sync.dma_start(out=out[n], in_=out2_sb[half][:, 0])
                elif n == NSC - 1:
                    nc.sync.dma_start(out=out[n], in_=out2_sb[half][:, 1])
                elif n % 2 == 1:
                    nc.sync.dma_start(
                        out=out[n - 1 : n + 1].rearrange("s p a b c -> p s a b c"),
                        in_=out2_sb[half],
                    )

            for it in range(NSC + 3):
                if 2 <= it <= NSC + 1:
                    emit_dcopy(it - 2)
                fp = emit_feat(it) if it < NSC else None
                if fp is not None:
                    fp(0)
                if 1 <= it <= NSC:
                    emit_transp(it - 1)
                if fp is not None:
                    fp(1)
                if 1 <= it <= NSC:
                    emit_mid_pe1(it - 1)
                if 1 <= it <= NSC:
                    emit_dmm(it - 1, 0)
                if 2 <= it <= NSC + 1:
                    emit_num(it - 2)
                if 1 <= it <= NSC:
                    emit_dmm(it - 1, 1)
                if 2 <= it <= NSC + 1:
                    emit_div(it - 2)
    nc.compile()
    return nc


def _get_nc():
    if "nc" not in _NC_CACHE:
        _NC_CACHE["nc"] = build_nc()
    return _NC_CACHE["nc"]


def shard_inputs(query, key, value, projection_matrix):
    bf = ml_dtypes.bfloat16
    q = np.transpose(query, (0, 2, 3, 1)).reshape(B * H, D, L)
    k = np.transpose(key, (0, 2, 3, 1)).reshape(B * H, D, L)
    qk = np.stack([q, k], axis=2).astype(bf)  # [BH, D, 2, L]
    vv = np.transpose(value, (0, 2, 1, 3)).reshape(B * H, NSC, NSUB, 128, D)
    vv = np.transpose(vv, (0, 3, 1, 2, 4))  # [BH, 128, NSC, NSUB, D]
    vvp = np.ones((B * H, 128, NSC, NSUB, DV), dtype=bf)
    vvp[..., 0:D] = vv.astype(bf)
    km = np.arange(128)
    consts = np.zeros((128, 256), dtype=bf)
    consts[:, 0:128] = (km[:, None] <= km[None, :]).astype(bf)
    consts[0:D, 128:256] = (projection_matrix.T * RATIO).astype(bf)
    in_maps = []
    for c in range(NCORES):
        sl = slice(c * NPAIR, (c + 1) * NPAIR)
        in_maps.append(
            {
                "qk2": np.ascontiguousarray(qk[sl]),
                "v": np.ascontiguousarray(vvp[sl]),
                "consts": consts,
            }
        )
    return in_maps


def unshard_output(results):
    o = np.stack([np.asarray(r["out"], dtype=np.float32) for r in results], axis=0)
    o = o.transpose(0, 3, 1, 4, 2, 5).reshape(B, H, L, D).transpose(0, 2, 1, 3)
    return np.ascontiguousarray(o)


def kernel(query, key, value, projection_matrix, _trace=False):
    nc = _get_nc()
    in_maps = shard_inputs(
        np.asarray(query, dtype=np.float32),
        np.asarray(key, dtype=np.float32),
        np.asarray(value, dtype=np.float32),
        np.asarray(projection_matrix, dtype=np.float32),
    )
    res = run_bass_kernel_spmd(nc, in_maps, core_ids=list(range(NCORES)), trace=_trace)
    out = unshard_output(res.results)
    if _trace:
        return out, res
    return out
